# revision 20
# baseline (speedup 1.0000x reference)
"""AttnBlockST Trainium2 kernel — fp8 DoubleRow edition.

Two SPMD phases on 8 NeuronCores:
  phase 1 (spatial): data-parallel over b*t (32 samples -> 4/core),
    attention over hw=1024 within each (bt, c, hw) sample.
  phase 2 (temporal): data-parallel over b*h*w (2048 -> 256/core),
    attention over t=16, 8 samples packed per 128-partition group with a
    block-diagonal logit mask.

Numerics plan (validated host-side, rel err ~0.009 vs 2e-2 gate):
  - x I/O in bf16, pre-scaled by 1024 host-side (GroupNorm is
    scale-invariant; the 1024 factor makes all fp8 operand scales fold
    into exact powers of two, final y comes back as 1024*(x+res)).
  - all matmuls in fp8e4 with DoubleRow perf mode (K=256 per pass):
    wq' = wq*c^-0.5*64, wk' = wk*16, wv' = wv*16, wo' = wo*16.
  - logits psum = 1024*logits -> exp(scale=2^-10) on ACT, fp8 out.
  - softmax denominators via an fp8 ones(0.25) matmul that broadcasts
    column sums across all 128 partitions; reciprocal on DVE; the
    normalization (and the fp8 O-cast scale 64) ride along for free.
  - v/o biases folded into bo host-side (sum(attn)==1), q/k biases
    added at the psum->fp8 cast (free scalar slot).
"""

import numpy as np
import ml_dtypes
from contextlib import ExitStack

import concourse.bass as bass
import concourse.mybir as mybir
import concourse.tile as tile
from concourse.bass_utils import run_bass_kernel_spmd

# ---- walrus workaround: split multi-wait final drain ----
from concourse.vector_clock import ScopedClock
from concourse.tile import TileContext


def _patched_drain_and_barrier(self, tick_clock, wait_clock):
    nc = self.nc
    drain_inst = nc.sync.drain()
    wait_clock.add_sem_waits(
        drain_inst.ins, ScopedClock({None: tick_clock.global_clock})
    )
    si = drain_inst.ins.sync_info
    if si is not None and len(si.on_wait) > 1:
        waits = list(si.on_wait)
        drain_inst.ins.sync_info = mybir.SyncInfo(
            on_wait=waits[:1], on_update=list(si.on_update)
        )
        for w in waits[1:]:
            n = nc.sync.nop(nofuse=True, hint="drain_wait_split")
            n.ins.sync_info = mybir.SyncInfo(on_wait=[w], on_update=[])
    nc.all_engine_barrier()
    assert self.sems is not None
    popped = nc._tile_sem_poison_stack.pop()
    assert popped is self._sem_poison
    nc.clear_and_free_semaphores(list(self.sems.allocated().values()))
    nc.all_engine_barrier()


TileContext._drain_and_barrier = _patched_drain_and_barrier

# ---- problem constants (hardcoded per spec) ----
B, C, T, H, W = 2, 512, 16, 32, 32
GROUPS = 32
EPS = 1e-6
N_CORES = 8
P = 128
CCH = C // P          # 4 channel chunks
GPC = GROUPS // CCH   # 8 groups per 128-channel chunk
GS = C // GROUPS      # 16 channels per group

L1 = H * W            # 1024 spatial positions
NS1 = (B * T) // N_CORES   # 4 samples per core, phase 1
LCH1 = L1 // P        # 8 position chunks

NT2 = 16              # temporal length
NS2 = (B * H * W) // N_CORES  # 256 samples per core, phase 2
HALF = NS2 // 2       # process in halves of 128 samples
F2 = HALF * NT2       # 2048 free columns per half
NB2 = F2 // 512       # 4 n-blocks of 512
NGRP = F2 // P        # 16 groups of 8 samples per half

# fp8 operand scales (exact powers of two; see module docstring)
XSCALE = 1024.0       # host pre-scale on x
SQ = 64.0             # wq * c^-0.5 * SQ
SK = 16.0             # wk * SK
SV = 16.0             # wv * SV
SO = 64.0             # O cast target scale
SW = 16.0             # wo * SW
EXPSCALE = 1.0 / (SQ * SK)      # 2^-10, exp() psum scale
ONESVAL = SV / SO               # 0.25, colsum lhsT value
EPS2 = EPS * XSCALE * XSCALE    # GN eps on scaled input

F32 = mybir.dt.float32
BF16 = mybir.dt.bfloat16
F8 = mybir.dt.float8e4
AX = mybir.AxisListType.X
AF = mybir.ActivationFunctionType
DR = mybir.MatmulPerfMode.DoubleRow


def _op():
    from concourse.alu_op_type import AluOpType
    return AluOpType


def _bcast_inner(ap, n):
    """View (P, F) access pattern as (P, F, n) with stride-0 inner dim."""
    return bass.AP(tensor=ap.tensor, offset=ap.offset, ap=list(ap.ap) + [[0, n]])


def _bcast_mid(ap, n):
    """View (P, F) access pattern as (P, n, F) with stride-0 middle dim."""
    return bass.AP(
        tensor=ap.tensor, offset=ap.offset,
        ap=[list(ap.ap[0])] + [[0, n]] + [list(d) for d in ap.ap[1:]],
    )


def _split_waits(nc, limit=1):
    """This walrus build rejects >1 sem wait on every ISA template tested
    (LDWEIGHTS, CTRL, ACT, DVE TensorScalar); hoist extra waits onto
    same-engine NoOps placed just before."""
    ctr = [0]
    for f in nc.m.functions:
        for b in f.blocks:
            new = []
            for ins in b.instructions:
                si = getattr(ins, "sync_info", None)
                waits = list(si.on_wait) if si is not None and si.on_wait else []
                lim = limit
                if len(waits) > lim:
                    for w in waits[lim:]:
                        ctr[0] += 1
                        new.append(mybir.InstNoOp(
                            name=f"wsplit-{ctr[0]}",
                            sync_info=mybir.SyncInfo(on_wait=[w], on_update=[]),
                            bass_nofuse=True,
                            engine=ins.engine,
                        ))
                    ins.sync_info = mybir.SyncInfo(
                        on_wait=waits[:lim], on_update=list(si.on_update)
                    )
                new.append(ins)
            b.instructions = new
    return nc


def _load_consts(nc, tc, ctx, wd, bd, gmask_d, bmask_d, mask_dt=F32, extra=()):
    const = ctx.enter_context(tc.tile_pool(name="const", bufs=1))
    w_sb = {}
    for n in wd:
        t = const.tile([P, CCH, C], F8, tag=n)
        nc.sync.dma_start(out=t, in_=wd[n].rearrange("(k p) o -> p k o", p=P))
        w_sb[n] = t
    b_sb = {}
    for n in bd:
        t = const.tile([P, CCH], F32, tag=n)
        nc.sync.dma_start(out=t, in_=bd[n][:, :])
        b_sb[n] = t
    gmask = const.tile([P, GPC], mask_dt, tag="gmask")
    nc.sync.dma_start(out=gmask, in_=gmask_d[:, :])
    bmask = const.tile([GPC, P], mask_dt, tag="bmask")
    nc.sync.dma_start(out=bmask, in_=bmask_d[:, :])
    ones8 = const.tile([P, 2, P], F8, tag="ones8")
    nc.vector.memset(ones8, ONESVAL)
    eps_t = const.tile([GPC, 1], F32, tag="eps")
    nc.vector.memset(eps_t, EPS2)
    out = [const, w_sb, b_sb, gmask, bmask, ones8, eps_t]
    for name, shape, dt in extra:
        t = const.tile(shape, dt, tag=name)
        out.append(t)
    return out


# ---------------------------------------------------------------- phase 1
def build_spatial(reps=1):
    nc = bass.Bass()
    xs = nc.dram_tensor("xs", [NS1, C, L1], BF16, kind="ExternalInput")
    ys = nc.dram_tensor("ys", [NS1, C, L1], BF16, kind="ExternalOutput")
    wd = {
        n: nc.dram_tensor(n, [C, C], F8, kind="ExternalInput")
        for n in ("wq", "wk", "wv", "wo")
    }
    bd = {
        n: nc.dram_tensor(n, [P, CCH], F32, kind="ExternalInput")
        for n in ("bq", "bk", "bo")
    }
    gmask_d = nc.dram_tensor("gmask", [P, GPC], F32, kind="ExternalInput")
    bmask_d = nc.dram_tensor("bmask", [GPC, P], F32, kind="ExternalInput")
    A = _op()

    with tile.TileContext(nc) as tc, ExitStack() as ctx:
        (const, w_sb, b_sb, gmask, bmask, ones8, eps_t) = _load_consts(
            nc, tc, ctx, wd, bd, gmask_d, bmask_d
        )
        stp = ctx.enter_context(tc.tile_pool(name="stats", bufs=3))
        xp = ctx.enter_context(tc.tile_pool(name="x", bufs=2))
        hp = ctx.enter_context(tc.tile_pool(name="h", bufs=2))
        qp = ctx.enter_context(tc.tile_pool(name="q", bufs=2))
        kp = ctx.enter_context(tc.tile_pool(name="k", bufs=2))
        vp = ctx.enter_context(tc.tile_pool(name="v", bufs=2))
        ep = ctx.enter_context(tc.tile_pool(name="e", bufs=2))
        op_ = ctx.enter_context(tc.tile_pool(name="o", bufs=2))
        rcp = ctx.enter_context(tc.tile_pool(name="rc", bufs=2))
        yp = ctx.enter_context(tc.tile_pool(name="y", bufs=3))
        psA = ctx.enter_context(tc.tile_pool(name="psA", bufs=4, space="PSUM"))
        psO = ctx.enter_context(tc.tile_pool(name="psO", bufs=3, space="PSUM"))
        psT = ctx.enter_context(tc.tile_pool(name="psT", bufs=1, space="PSUM"))

        for i_rep in range(reps * NS1):
            i = i_rep % NS1
            x_sb = xp.tile([P, CCH, L1], BF16)
            nc.sync.dma_start(out=x_sb, in_=xs[i].rearrange("(k p) l -> p k l", p=P))

            # ---- GroupNorm -> h (fp8) ----
            h8 = hp.tile([P, CCH, L1], F8, tag="h")
            for k in range(CCH):
                xc = x_sb[:, k, :]
                st = stp.tile([P, 2, 6], F32, tag="bnst")
                nc.vector.bn_stats(out=st[:, 0, :], in_=xc[:, 0:512])
                nc.vector.bn_stats(out=st[:, 1, :], in_=xc[:, 512:1024])
                mv = stp.tile([P, 2], F32, tag="mv")
                nc.vector.bn_aggr(out=mv, in_=st)
                me = stp.tile([P, 2], F32, tag="me")
                nc.vector.tensor_copy(out=me[:, 0:1], in_=mv[:, 0:1])
                m2 = stp.tile([P, 1], F32, tag="m2")
                nc.vector.tensor_mul(out=m2, in0=mv[:, 0:1], in1=mv[:, 0:1])
                nc.vector.tensor_add(out=me[:, 1:2], in0=mv[:, 1:2], in1=m2)
                gs_ps = psT.tile([GPC, 2], F32, tag="pt")
                nc.tensor.matmul(out=gs_ps, lhsT=gmask, rhs=me, start=True, stop=True)
                gs = stp.tile([GPC, 2], F32, tag="gs")
                nc.vector.tensor_copy(out=gs, in_=gs_ps)
                var = stp.tile([GPC, 1], F32, tag="var")
                nc.vector.tensor_mul(out=var, in0=gs[:, 0:1], in1=gs[:, 0:1])
                var2 = stp.tile([GPC, 1], F32, tag="var2")
                nc.vector.tensor_sub(out=var2, in0=gs[:, 1:2], in1=var)
                # rstd = exp(-0.5*ln(var+eps)): keeps ACT on one table
                # (natural_log_exp_and_others holds ln+exp+identity)
                lnv = stp.tile([GPC, 1], F32, tag="lnv")
                nc.scalar.activation(out=lnv, in_=var2, func=AF.Ln, bias=eps_t)
                ab = stp.tile([GPC, 2], F32, tag="ab")
                nc.scalar.activation(out=ab[:, 0:1], in_=lnv, func=AF.Exp, scale=-0.5)
                nc.vector.scalar_tensor_tensor(
                    out=ab[:, 1:2], in0=gs[:, 0:1], scalar=-1.0, in1=ab[:, 0:1],
                    op0=A.mult, op1=A.mult,
                )
                abc_ps = psT.tile([P, 2], F32, tag="pt")
                nc.tensor.matmul(out=abc_ps, lhsT=bmask, rhs=ab, start=True, stop=True)
                abc = stp.tile([P, 2], F32, tag="abc")
                nc.vector.tensor_copy(out=abc, in_=abc_ps)
                nc.vector.tensor_scalar(
                    out=h8[:, k, :], in0=xc,
                    scalar1=abc[:, 0:1], scalar2=abc[:, 1:2],
                    op0=A.mult, op1=A.add,
                )

            # ---- q, k projections (c-major, fp8 DoubleRow) ----
            # t-loop outer so each lhsT load serves both nb matmuls
            q8t = qp.tile([P, CCH, L1], F8, tag="q")
            k8t = kp.tile([P, CCH, L1], F8, tag="k")
            for wname, dst, bname in (("wq", q8t, "bq"), ("wk", k8t, "bk")):
                for m in range(CCH):
                    pss = [psA.tile([P, 512], F32, tag="mm", name=f"mm{_i}") for _i in range(2)]
                    for t in range(2):
                        for nb in range(2):
                            nc.tensor.matmul(
                                out=pss[nb],
                                lhsT=w_sb[wname][:, 2 * t:2 * t + 2, m * P:(m + 1) * P],
                                rhs=h8[:, 2 * t:2 * t + 2, nb * 512:(nb + 1) * 512],
                                start=(t == 0), stop=(t == 1), perf_mode=DR,
                            )
                    for nb in range(2):
                        nc.scalar.activation(
                            out=dst[:, m, nb * 512:(nb + 1) * 512], in_=pss[nb],
                            func=AF.Identity, bias=b_sb[bname][:, m:m + 1],
                        )

            # ---- v^T (positions on partitions; bias folded into bo) ----
            vT8 = vp.tile([P, LCH1, C], F8, tag="v")
            for m in range(LCH1):
                ps = psA.tile([P, 512], F32, tag="mm")
                for t in range(2):
                    nc.tensor.matmul(
                        out=ps,
                        lhsT=h8[:, 2 * t:2 * t + 2, m * P:(m + 1) * P],
                        rhs=w_sb["wv"][:, 2 * t:2 * t + 2, :],
                        start=(t == 0), stop=(t == 1), perf_mode=DR,
                    )
                nc.vector.tensor_copy(out=vT8[:, m, :], in_=ps)

            # ---- S^T = k^T q blocks -> exp -> E (fp8), k on partitions ----
            e8 = ep.tile([P, LCH1, L1], F8, tag="e")
            for j in range(LCH1):
                pss = [psA.tile([P, 512], F32, tag="mm", name=f"mm{_i}") for _i in range(2)]
                for t in range(2):
                    for nb in range(2):
                        nc.tensor.matmul(
                            out=pss[nb],
                            lhsT=k8t[:, 2 * t:2 * t + 2, j * P:(j + 1) * P],
                            rhs=q8t[:, 2 * t:2 * t + 2, nb * 512:(nb + 1) * 512],
                            start=(t == 0), stop=(t == 1), perf_mode=DR,
                        )
                for nb in range(2):
                    nc.scalar.activation(
                        out=e8[:, j, nb * 512:(nb + 1) * 512], in_=pss[nb],
                        func=AF.Exp, scale=EXPSCALE,
                    )

            # ---- column sums broadcast to all partitions, reciprocal ----
            rc = rcp.tile([P, L1], F32, tag="rc")
            psrb = [psO.tile([P, 512], F32, tag="o", name=f"rb{_i}") for _i in range(2)]
            for u in range(4):
                for nb in range(2):
                    nc.tensor.matmul(
                        out=psrb[nb],
                        lhsT=ones8,
                        rhs=e8[:, 2 * u:2 * u + 2, nb * 512:(nb + 1) * 512],
                        start=(u == 0), stop=(u == 3), perf_mode=DR,
                    )
            for nb in range(2):
                nc.vector.reciprocal_approx_fast(
                    out=rc[:, nb * 512:(nb + 1) * 512], in_=psrb[nb]
                )

            # ---- O = v E (unnormalized), cast scales by rc ----
            o8 = op_.tile([P, CCH, L1], F8, tag="o")
            for m in range(CCH):
                pss = [psO.tile([P, 512], F32, tag="o", name=f"o{_i}") for _i in range(2)]
                for u in range(4):
                    for nb in range(2):
                        nc.tensor.matmul(
                            out=pss[nb],
                            lhsT=vT8[:, 2 * u:2 * u + 2, m * P:(m + 1) * P],
                            rhs=e8[:, 2 * u:2 * u + 2, nb * 512:(nb + 1) * 512],
                            start=(u == 0), stop=(u == 3), perf_mode=DR,
                        )
                for nb in range(2):
                    nc.vector.tensor_tensor(
                        out=o8[:, m, nb * 512:(nb + 1) * 512],
                        in0=pss[nb], in1=rc[:, nb * 512:(nb + 1) * 512], op=A.mult,
                    )

            # ---- y = wo O + bo' + x -> ys (bf16, still 1024x scaled) ----
            for m in range(CCH):
                y_sb = yp.tile([P, L1], BF16, tag="y")
                pss = [psA.tile([P, 512], F32, tag="mm", name=f"mm{_i}") for _i in range(2)]
                for t in range(2):
                    for nb in range(2):
                        nc.tensor.matmul(
                            out=pss[nb],
                            lhsT=w_sb["wo"][:, 2 * t:2 * t + 2, m * P:(m + 1) * P],
                            rhs=o8[:, 2 * t:2 * t + 2, nb * 512:(nb + 1) * 512],
                            start=(t == 0), stop=(t == 1), perf_mode=DR,
                        )
                for nb in range(2):
                    nc.vector.scalar_tensor_tensor(
                        out=y_sb[:, nb * 512:(nb + 1) * 512], in0=pss[nb],
                        scalar=b_sb["bo"][:, m:m + 1],
                        in1=x_sb[:, m, nb * 512:(nb + 1) * 512],
                        op0=A.add, op1=A.add,
                    )
                nc.sync.dma_start(out=ys[i, m * P:(m + 1) * P, :], in_=y_sb)
    return nc


# ---------------------------------------------------------------- phase 2
def build_temporal(reps=1):
    nc = bass.Bass()
    xt = nc.dram_tensor("xt", [C, NS2 * NT2], BF16, kind="ExternalInput")
    yt = nc.dram_tensor("yt", [C, NS2 * NT2], BF16, kind="ExternalOutput")
    wd = {
        n: nc.dram_tensor(n, [C, C], F8, kind="ExternalInput")
        for n in ("wq", "wk", "wv", "wo")
    }
    bd = {
        n: nc.dram_tensor(n, [P, CCH], F32, kind="ExternalInput")
        for n in ("bq", "bk", "bo")
    }
    gmask_d = nc.dram_tensor("gmask", [P, GPC], BF16, kind="ExternalInput")
    bmask_d = nc.dram_tensor("bmask", [GPC, P], BF16, kind="ExternalInput")
    blkmask_d = nc.dram_tensor("blkmask", [P, P], F32, kind="ExternalInput")
    A = _op()
    NN = HALF  # samples per half

    with tile.TileContext(nc) as tc, ExitStack() as ctx:
        (const, w_sb, b_sb, gmask, bmask, ones8, eps_t) = _load_consts(
            nc, tc, ctx, wd, bd, gmask_d, bmask_d, mask_dt=BF16
        )
        blkmask = const.tile([P, P], F32, tag="blkmask")
        nc.sync.dma_start(out=blkmask, in_=blkmask_d[:, :])

        stp = ctx.enter_context(tc.tile_pool(name="stats", bufs=3))
        xp = ctx.enter_context(tc.tile_pool(name="x", bufs=2))
        sqp = ctx.enter_context(tc.tile_pool(name="sq", bufs=2))
        tmpp = ctx.enter_context(tc.tile_pool(name="tmp", bufs=2))
        hp = ctx.enter_context(tc.tile_pool(name="h", bufs=2))
        qp = ctx.enter_context(tc.tile_pool(name="q", bufs=2))
        kp = ctx.enter_context(tc.tile_pool(name="k", bufs=2))
        vp = ctx.enter_context(tc.tile_pool(name="v", bufs=2))
        ep = ctx.enter_context(tc.tile_pool(name="e", bufs=2))
        op_ = ctx.enter_context(tc.tile_pool(name="o", bufs=2))
        rcp = ctx.enter_context(tc.tile_pool(name="rc", bufs=2))
        yp = ctx.enter_context(tc.tile_pool(name="y", bufs=3))
        psA = ctx.enter_context(tc.tile_pool(name="psA", bufs=3, space="PSUM"))
        psS = ctx.enter_context(tc.tile_pool(name="psS", bufs=2, space="PSUM"))
        psO = ctx.enter_context(tc.tile_pool(name="psO", bufs=2, space="PSUM"))
        psT = ctx.enter_context(tc.tile_pool(name="psT", bufs=1, space="PSUM"))

        xr = xt.rearrange("(k p) f -> p k f", p=P)
        yr = yt.rearrange("(k p) f -> p k f", p=P)

        for ih_rep in range(reps * 2):
            ih = ih_rep % 2
            f0 = ih * F2
            x_sb = xp.tile([P, CCH, F2], BF16)
            nc.sync.dma_start(out=x_sb, in_=xr[:, :, f0:f0 + F2])

            # ---- GroupNorm over (16c x 16t) per sample -> h (fp8) ----
            h8 = hp.tile([P, CCH, F2], F8, tag="h")
            for k in range(CCH):
                xc = x_sb[:, k, :]
                xc3 = x_sb[:, k, :].rearrange("p (n t) -> p n t", t=NT2)
                sq = sqp.tile([P, F2], BF16, tag="sq")
                nc.vector.tensor_mul(out=sq, in0=xc, in1=xc)
                me = stp.tile([P, 2, NN], BF16, tag="me2")
                with nc.allow_low_precision(reason="16-wide t-sums; bf16 ample"):
                    nc.vector.reduce_sum(out=me[:, 0, :], in_=xc3, axis=AX)
                    nc.vector.reduce_sum(
                        out=me[:, 1, :],
                        in_=sq.rearrange("p (n t) -> p n t", t=NT2), axis=AX,
                    )
                gs_ps = psT.tile([GPC, 2, NN], F32, tag="pt")
                nc.tensor.matmul(
                    out=gs_ps.rearrange("g a n -> g (a n)"),
                    lhsT=gmask, rhs=me.rearrange("p a n -> p (a n)"),
                    start=True, stop=True,
                )
                gs = stp.tile([GPC, 2, NN], F32, tag="gs2")
                nc.vector.tensor_copy(out=gs, in_=gs_ps)
                var = stp.tile([GPC, NN], F32, tag="var2a")
                nc.vector.tensor_mul(out=var, in0=gs[:, 0, :], in1=gs[:, 0, :])
                var2 = stp.tile([GPC, NN], F32, tag="var2b")
                nc.vector.tensor_sub(out=var2, in0=gs[:, 1, :], in1=var)
                # rstd = exp(-0.5*ln(var+eps)): single ACT table
                lnv = stp.tile([GPC, NN], F32, tag="lnv2")
                nc.scalar.activation(out=lnv, in_=var2, func=AF.Ln, bias=eps_t)
                ab = stp.tile([GPC, 2, NN], BF16, tag="ab2")
                nc.scalar.activation(out=ab[:, 0, :], in_=lnv, func=AF.Exp, scale=-0.5)
                nc.vector.scalar_tensor_tensor(
                    out=ab[:, 1, :], in0=gs[:, 0, :], scalar=-1.0, in1=ab[:, 0, :],
                    op0=A.mult, op1=A.mult,
                )
                abc_ps = psT.tile([P, 2, NN], F32, tag="pt")
                nc.tensor.matmul(
                    out=abc_ps.rearrange("p a n -> p (a n)"),
                    lhsT=bmask, rhs=ab.rearrange("g a n -> g (a n)"),
                    start=True, stop=True,
                )
                abc = stp.tile([P, 2, NN], BF16, tag="abc2")
                nc.vector.tensor_copy(out=abc, in_=abc_ps)
                tmp = tmpp.tile([P, F2], BF16, tag="tmp")
                nc.vector.tensor_tensor(
                    out=tmp.rearrange("p (n t) -> p n t", t=NT2),
                    in0=xc3, in1=_bcast_inner(abc[:, 0, :], NT2), op=A.mult,
                )
                nc.vector.tensor_tensor(
                    out=h8[:, k, :].rearrange("p (n t) -> p n t", t=NT2),
                    in0=tmp.rearrange("p (n t) -> p n t", t=NT2),
                    in1=_bcast_inner(abc[:, 1, :], NT2), op=A.add,
                )

            # ---- q, k projections (fp8 DoubleRow) ----
            q8t = qp.tile([P, CCH, F2], F8, tag="q")
            k8t = kp.tile([P, CCH, F2], F8, tag="k")
            for wname, dst, bname in (("wq", q8t, "bq"), ("wk", k8t, "bk")):
                for m in range(CCH):
                    for nbp in range(NB2 // 2):
                        pss = [psA.tile([P, 512], F32, tag="mm", name=f"mm{_i}") for _i in range(2)]
                        for t in range(2):
                            for i2 in range(2):
                                nb = 2 * nbp + i2
                                nc.tensor.matmul(
                                    out=pss[i2],
                                    lhsT=w_sb[wname][:, 2 * t:2 * t + 2, m * P:(m + 1) * P],
                                    rhs=h8[:, 2 * t:2 * t + 2, nb * 512:(nb + 1) * 512],
                                    start=(t == 0), stop=(t == 1), perf_mode=DR,
                                )
                        for i2 in range(2):
                            nb = 2 * nbp + i2
                            nc.scalar.activation(
                                out=dst[:, m, nb * 512:(nb + 1) * 512], in_=pss[i2],
                                func=AF.Identity, bias=b_sb[bname][:, m:m + 1],
                            )

            # ---- v^T per group (bias folded into bo) ----
            vT8 = vp.tile([P, NGRP, C], F8, tag="v")
            for g in range(NGRP):
                ps = psA.tile([P, 512], F32, tag="mm")
                for t in range(2):
                    nc.tensor.matmul(
                        out=ps,
                        lhsT=h8[:, 2 * t:2 * t + 2, g * P:(g + 1) * P],
                        rhs=w_sb["wv"][:, 2 * t:2 * t + 2, :],
                        start=(t == 0), stop=(t == 1), perf_mode=DR,
                    )
                nc.scalar.activation(out=vT8[:, g, :], in_=ps, func=AF.Copy)

            # ---- S^T per 8-sample group + mask -> exp -> E (fp8) ----
            e8 = ep.tile([P, NGRP, P], F8, tag="e")
            for g in range(NGRP):
                c0 = g * P
                ps = psS.tile([P, P], F32, tag="s")
                for t in range(2):
                    nc.tensor.matmul(
                        out=ps,
                        lhsT=k8t[:, 2 * t:2 * t + 2, c0:c0 + P],
                        rhs=q8t[:, 2 * t:2 * t + 2, c0:c0 + P],
                        start=(t == 0), stop=(t == 1), perf_mode=DR,
                    )
                nc.vector.tensor_add(out=ps, in0=ps, in1=blkmask)
                nc.scalar.activation(
                    out=e8[:, g, :], in_=ps, func=AF.Exp, scale=EXPSCALE,
                )

            # ---- column sums (broadcast), reciprocal ----
            rc = rcp.tile([P, F2], F32, tag="rc")
            for q4 in range(NGRP // 4):
                ps = psO.tile([P, 512], F32, tag="o")
                nc.tensor.matmul(
                    out=ps, lhsT=ones8[:, 0, :],
                    rhs=e8[:, 4 * q4:4 * q4 + 4, :],
                    start=True, stop=True,
                )
                nc.vector.reciprocal_approx_fast(
                    out=rc[:, q4 * 512:(q4 + 1) * 512], in_=ps
                )

            # ---- O per group, cast scales by rc ----
            o8 = op_.tile([P, CCH, F2], F8, tag="o")
            for g in range(NGRP):
                c0 = g * P
                ps = psO.tile([P, 512], F32, tag="o")
                for m in range(CCH):
                    nc.tensor.matmul(
                        out=ps[:, m * P:(m + 1) * P],
                        lhsT=vT8[:, g, m * P:(m + 1) * P],
                        rhs=e8[:, g, :],
                        start=True, stop=True,
                    )
                nc.vector.tensor_tensor(
                    out=o8[:, :, c0:c0 + P],
                    in0=ps.rearrange("p (m q) -> p m q", q=P),
                    in1=_bcast_mid(rc[:, c0:c0 + P], CCH),
                    op=A.mult,
                )

            # ---- y = wo O + bo' + x -> yt ----
            for m in range(CCH):
                y_sb = yp.tile([P, F2], BF16, tag="y")
                for nbp in range(NB2 // 2):
                    pss = [psA.tile([P, 512], F32, tag="mm", name=f"mm{_i}") for _i in range(2)]
                    for t in range(2):
                        for i2 in range(2):
                            nb = 2 * nbp + i2
                            nc.tensor.matmul(
                                out=pss[i2],
                                lhsT=w_sb["wo"][:, 2 * t:2 * t + 2, m * P:(m + 1) * P],
                                rhs=o8[:, 2 * t:2 * t + 2, nb * 512:(nb + 1) * 512],
                                start=(t == 0), stop=(t == 1), perf_mode=DR,
                            )
                    for i2 in range(2):
                        nb = 2 * nbp + i2
                        nc.vector.scalar_tensor_tensor(
                            out=y_sb[:, nb * 512:(nb + 1) * 512], in0=pss[i2],
                            scalar=b_sb["bo"][:, m:m + 1],
                            in1=x_sb[:, m, nb * 512:(nb + 1) * 512],
                            op0=A.add, op1=A.add,
                        )
                nc.sync.dma_start(out=yr[:, m, f0:f0 + F2], in_=y_sb)
    return nc


# ---------------------------------------------------------------- host side
F8NP = ml_dtypes.float8_e4m3
BFNP = ml_dtypes.bfloat16


def _fold_weights(w, b, gamma, beta, scale):
    """GN affine folded into conv, then fp8 scale: returns
    (lhsT fp8 (c,o) scaled, bias f32 (128,4) scaled)."""
    w = np.asarray(w, np.float32)
    b = np.asarray(b, np.float32)
    gamma = np.asarray(gamma, np.float32)
    beta = np.asarray(beta, np.float32)
    w_eff = w * gamma[None, :] * scale
    b_eff = (b + w @ beta) * scale
    wT = np.ascontiguousarray(w_eff.T).astype(F8NP)
    bb = np.ascontiguousarray(b_eff.reshape(CCH, P).T)
    return wT, bb, b_eff / scale  # unscaled eff bias for bo folding


def _consts():
    gmask1 = np.zeros((P, GPC), np.float32)
    for p in range(P):
        gmask1[p, p // GS] = 1.0 / GS  # spatial: avg of 16 channel stats
    gmask2 = np.zeros((P, GPC), np.float32)
    for p in range(P):
        gmask2[p, p // GS] = 1.0 / (GS * NT2)  # temporal: full group sum / 256
    bmask = np.zeros((GPC, P), np.float32)
    for p in range(P):
        bmask[p // GS, p] = 1.0
    blk = np.full((P, P), -1e9, np.float32)
    for n in range(P // NT2):
        blk[n * NT2:(n + 1) * NT2, n * NT2:(n + 1) * NT2] = 0.0
    return gmask1, gmask2, bmask, blk


def _prep_phase(inputs, sfx, gamma, beta):
    cs = float(C) ** -0.5
    wq8, bq, _ = _fold_weights(inputs[f"wq_{sfx}"], inputs[f"bq_{sfx}"],
                               gamma, beta, cs * SQ)
    wk8, bk, _ = _fold_weights(inputs[f"wk_{sfx}"], inputs[f"bk_{sfx}"],
                               gamma, beta, SK)
    wv8, _, bv_eff = _fold_weights(inputs[f"wv_{sfx}"], inputs[f"bv_{sfx}"],
                                   gamma, beta, SV)
    wo = np.asarray(inputs[f"wo_{sfx}"], np.float32)
    bo = np.asarray(inputs[f"bo_{sfx}"], np.float32)
    wo8 = np.ascontiguousarray((wo * SW).T).astype(F8NP)
    bo_eff = XSCALE * (bo + wo @ bv_eff)
    bo_t = np.ascontiguousarray(bo_eff.reshape(CCH, P).T).astype(np.float32)
    return dict(wq=wq8, wk=wk8, wv=wv8, wo=wo8, bq=bq, bk=bk, bo=bo_t)


_CACHE = {}


def kernel(**inputs):
    x = np.asarray(inputs["x"], np.float32)
    gmask1, gmask2, bmask, blk = _consts()

    p1 = _prep_phase(inputs, "s", inputs["gamma_s"], inputs["beta_s"])
    p2 = _prep_phase(inputs, "t", inputs["gamma_t"], inputs["beta_t"])

    if "nc1" not in _CACHE:
        _CACHE["nc1"] = _split_waits(build_spatial())
        _CACHE["nc2"] = _split_waits(build_temporal())
    nc1, nc2 = _CACHE["nc1"], _CACHE["nc2"]

    # ---- phase 1: spatial over (b t) ----
    x1024 = (x * XSCALE).astype(BFNP)
    xs = np.ascontiguousarray(
        x1024.transpose(0, 2, 1, 3, 4).reshape(B * T, C, L1)
    )
    common1 = dict(gmask=gmask1, bmask=bmask, **p1)
    in_maps1 = [
        dict(xs=np.ascontiguousarray(xs[i * NS1:(i + 1) * NS1]), **common1)
        for i in range(N_CORES)
    ]
    _CACHE["in_maps1"] = in_maps1
    r1 = run_bass_kernel_spmd(nc1, in_maps1, core_ids=list(range(N_CORES)),
                              **_CACHE.get("run_kwargs", {}))
    ys = np.concatenate([r1.results[i]["ys"] for i in range(N_CORES)], axis=0)
    _CACHE["last_r1"] = r1

    # ---- phase 2: temporal over (b h w); input still 1024x bf16 ----
    x2 = ys.reshape(B, T, C, H, W).transpose(0, 3, 4, 2, 1)  # (b,h,w,c,t)
    x2 = np.ascontiguousarray(x2).reshape(B * H * W, C, NT2)
    common2 = dict(gmask=gmask2.astype(BFNP), bmask=bmask.astype(BFNP),
                   blkmask=blk, **p2)
    in_maps2 = []
    for i in range(N_CORES):
        shard = x2[i * NS2:(i + 1) * NS2]          # (256, 512, 16)
        xt = np.ascontiguousarray(shard.transpose(1, 0, 2)).reshape(C, NS2 * NT2)
        in_maps2.append(dict(xt=xt, **common2))
    _CACHE["in_maps2"] = in_maps2
    r2 = run_bass_kernel_spmd(nc2, in_maps2, core_ids=list(range(N_CORES)),
                              **_CACHE.get("run_kwargs", {}))
    _CACHE["last_r2"] = r2

    out = np.empty((B * H * W, C, NT2), np.float32)
    for i in range(N_CORES):
        yt = r2.results[i]["yt"].astype(np.float32).reshape(C, NS2, NT2)
        out[i * NS2:(i + 1) * NS2] = yt.transpose(1, 0, 2)
    out = out.reshape(B, H, W, C, NT2).transpose(0, 3, 4, 1, 2) * (1.0 / XSCALE)
    return np.ascontiguousarray(out)
